# revision 7
# baseline (speedup 1.0000x reference)
"""Distributed 1D attention kernel for Trainium2 (8 NeuronCores), v2.

Problem: x [4,256,2048], y [4,256,2048] ->
  q = Wq@x, k = Wk@y, v = Wv@y  (per-head d=128, H=8 heads)
  out = Wo @ concat_h(softmax(q^T k / sqrt(128)) @ v^T)   -> [4,128,2048]

Sharding: core = 2*b + g where b in [0,4) is the batch and g in {0,1} picks
heads [4g, 4g+4). Each core computes its 4 (b,h) attention pairs; the host
sums the two per-batch partials and transposes.

Architecture (vs the v1 baseline):
- Wo is folded into the V projection on the host: VW_h = Wo_h @ Wv_h, so the
  device projects y straight to VW_h @ y and never runs a Wo stage.
- AV runs transposed: po[x128, 129] = sum_y E[y, x]^T (lhsT) @ VWT[y, 0:129]
  where VWT col 128 is ones, so the softmax denominator lands in po[:, 128]
  for free. That removes all pair-add/tree reductions and denominator
  matmuls of v1, and the reciprocal shrinks to [128, 2].
- Normalization + head-sum run as tensor_scalar / scalar_tensor_tensor on
  DVE with the per-partition scalar 1/den, accumulated in f32 into ACC,
  DMA'd out as outT [N, 128] f32 (host transposes + sums core pairs).
- The kernel is ScalarE-bound (16.8M exp elements per core through the ACT
  engine at ~1.1us per [128,1024] activation). Everything else is paced to
  keep that stream gapless: a two-deep software pipeline A(s)..B(s-2), and
  projection work units pulled one per logits group inside phase_a so their
  PSUM-cast WAR stalls hide under the ACT-paced stream.
"""

import sys

if "/opt/trn_rl_repo" not in sys.path:
    sys.path.insert(0, "/opt/trn_rl_repo")

import numpy as np
import ml_dtypes


def _install_ntff_shim():
    """antenv.axon_hooks is absent from this image, which crashes
    run_bass_kernel_spmd(trace=True). Recreate it from the hook factory
    that trn_agent_boot ships."""
    import types

    if "antenv.axon_hooks" in sys.modules:
        return
    mod = types.ModuleType("antenv.axon_hooks")
    _hook = [None]
    mod.set_axon_ntff_profile_hook = lambda h: _hook.__setitem__(0, h)
    mod.get_axon_ntff_profile_hook = lambda: _hook[0]
    sys.modules["antenv.axon_hooks"] = mod
    try:
        import antenv

        antenv.axon_hooks = mod
    except ImportError:
        pass
    try:
        from trn_agent_boot.trn_boot import _ntff_profile_via_ctypes

        mod.set_axon_ntff_profile_hook(
            _ntff_profile_via_ctypes("/opt/axon/libaxon_pjrt.so")
        )
    except Exception:
        pass


_install_ntff_shim()

import concourse.bass as bass
import concourse.mybir as mybir
import concourse.tile as tile
from concourse.bass_utils import run_bass_kernel_spmd

B, C, N, H, D = 4, 256, 2048, 8, 128
HPC = H // 2  # heads per core
NCORES = 8
BF = mybir.dt.bfloat16
F32 = mybir.dt.float32
NYT = N // 128  # 16 y tiles
NXB = N // 512  # 4 x blocks
SCALE = 1.0 / float(np.sqrt(D))

LAST_EXEC_NS = None
LAST_RESULTS = None


def _split_multi_waits(nc):
    """This walrus build accepts at most ONE sync wait per instruction;
    Tile's semaphore assignment attaches several. Hoist the extras into
    standalone event-semaphore instructions on the same engine."""
    ctr = 0
    for fn in nc.m.functions:
        for blk in fn.blocks:
            new_list = []
            changed = False
            for inst in blk.instructions:
                si = inst.sync_info
                if si is not None and len(si.on_wait) > 1:
                    waits = list(si.on_wait)
                    ups = list(si.on_update)
                    for w in waits[:-1]:
                        ev = mybir.InstEventSemaphore(
                            name=f"waitsplit-{ctr}", ins=[], outs=[]
                        )
                        ctr += 1
                        ev.engine = inst.engine
                        ev.sync_info = mybir.SyncInfo(on_wait=[w], on_update=[])
                        new_list.append(ev)
                    inst.sync_info = mybir.SyncInfo(on_wait=[waits[-1]], on_update=ups)
                    changed = True
                new_list.append(inst)
            if changed:
                blk.instructions = new_list
    return ctr


def _build_nc():
    nc = bass.Bass("TRN2", target_bir_lowering=False, debug=False)

    xb = nc.dram_tensor("xb", [C, N], BF, kind="ExternalInput")
    yb = nc.dram_tensor("yb", [C, N], BF, kind="ExternalInput")
    # wpack = [WKT | WQT | WVWT] along the output dim, [c, 3*hd]
    wpack = nc.dram_tensor("wpack", [C, 3 * HPC * D], BF, kind="ExternalInput")
    # transposed partial output: row x, col o; host sums cores + transposes
    out = nc.dram_tensor("out", [N, D], F32, kind="ExternalOutput")

    EXPF = mybir.ActivationFunctionType.Exp

    with tile.TileContext(nc) as tc:
        with (
            tc.tile_pool(name="w", bufs=1) as wpool,
            tc.tile_pool(name="big", bufs=1) as bigpool,
            tc.tile_pool(name="e", bufs=24) as epool,
            tc.tile_pool(name="rc", bufs=4) as rcpool,
            tc.tile_pool(name="pl", bufs=2, space="PSUM") as plpool,
            tc.tile_pool(name="po", bufs=2, space="PSUM") as popool,
            tc.tile_pool(name="pr", bufs=2, space="PSUM") as prpool,
        ):
            # ---- input loads ----------------------------------------------
            xr = xb.rearrange("(kt p) n -> p kt n", p=128)
            yr = yb.rearrange("(kt p) n -> p kt n", p=128)
            wpr = wpack.rearrange("(kt p) m -> p kt m", p=128)
            # act-table preload for Exp while DMAs run
            TMPP = wpool.tile([128, 32], BF, tag="TMPP")
            nc.gpsimd.memset(TMPP[:, 0:16], 0.0)
            nc.scalar.activation(TMPP[:, 16:32], TMPP[:, 0:16], EXPF)
            WP = wpool.tile([128, 2, 3 * HPC * D], BF, tag="WP")
            WKT = WP[:, :, 0 : HPC * D]
            WQT = WP[:, :, HPC * D : 2 * HPC * D]
            WVWT = WP[:, :, 2 * HPC * D : 3 * HPC * D]
            Y = bigpool.tile([128, 2, N], BF, tag="Y")
            X = bigpool.tile([128, 2, N], BF, tag="X")
            # order: K path (WKT, Y) and the slot-0 Q slice first, rest after
            nc.sync.dma_start(
                WP[:, :, 0 : 2 * HPC * D], wpr[:, :, 0 : 2 * HPC * D]
            )
            nc.sync.dma_start(Y[:, :, :], yr[:, :, :])
            nc.sync.dma_start(X[:, :, 0:512], xr[:, :, 0:512])
            nc.sync.dma_start(X[:, :, 512:N], xr[:, :, 512:N])
            nc.sync.dma_start(
                WP[:, :, 2 * HPC * D : 3 * HPC * D],
                wpr[:, :, 2 * HPC * D : 3 * HPC * D],
            )

            ONES = wpool.tile([128, 128], BF, tag="ONES")
            nc.gpsimd.memset(ONES[:], 1.0)
            # HAM warm-up: keep the PE clock-gate open while input DMAs run,
            # so the first real matmuls start at 2.4 GHz instead of 1.2.
            WARM = plpool.tile([128, 1024], F32, tag="pl", name="warm")
            for _wi in range(48):
                nc.tensor.matmul(
                    WARM[:, 0:128], ONES[:], ONES[:], start=True, stop=True
                )

            Q = bigpool.tile([128, HPC, N], BF, tag="Q")
            K = bigpool.tile([128, HPC, N], BF, tag="K")
            # VW^T with a ones column: [y, yt, h, 0:128]=VW, [.., 128]=1
            VWT = bigpool.tile([128, NYT, HPC, D + 1], BF, tag="VWT")
            nc.gpsimd.memset(VWT[:, :, :, 128:129], 1.0)
            # f32 accumulator [x128, xblk, xsub, o]
            ACC = bigpool.tile([128, NXB, 4, D], F32, tag="ACC")

            def proj_k(h, nb_lo, nb_hi):
                hs = slice(h * 128, (h + 1) * 128)
                for nb in range(nb_lo, nb_hi):
                    ns = slice(nb * 512, (nb + 1) * 512)
                    pk = prpool.tile([128, 512], F32, tag="pr", name=f"pk_{h}_{nb}")
                    nc.tensor.matmul(
                        pk[:], WKT[:, 0, hs], Y[:, 0, ns], start=True, stop=False
                    )
                    nc.tensor.matmul(
                        pk[:], WKT[:, 1, hs], Y[:, 1, ns], start=False, stop=True
                    )
                    nc.vector.tensor_copy(K[:, h, ns], pk[:])

            def proj_q(h, nb_lo, nb_hi):
                hs = slice(h * 128, (h + 1) * 128)
                for nb in range(nb_lo, nb_hi):
                    ns = slice(nb * 512, (nb + 1) * 512)
                    ps = prpool.tile([128, 512], F32, tag="pr", name=f"pq_{h}_{nb}")
                    nc.tensor.matmul(
                        ps[:], WQT[:, 0, hs], X[:, 0, ns], start=True, stop=False
                    )
                    nc.tensor.matmul(
                        ps[:], WQT[:, 1, hs], X[:, 1, ns], start=False, stop=True
                    )
                    nc.vector.tensor_copy(Q[:, h, ns], ps[:])

            def proj_vw(yt0, yt1):
                for yt in range(yt0, yt1):
                    ys = slice(yt * 128, (yt + 1) * 128)
                    pv = prpool.tile(
                        [128, 4, 128], F32, tag="pr", name=f"pv_{yt}"
                    )
                    nc.tensor.matmul(
                        pv[:], Y[:, 0, ys], WVWT[:, 0, :], start=True, stop=False
                    )
                    nc.tensor.matmul(
                        pv[:], Y[:, 1, ys], WVWT[:, 1, :], start=False, stop=True
                    )
                    nc.vector.tensor_copy(VWT[:, yt, :, 0:128], pv[:])

            # ---- attention slots, software-pipelined two deep ---------------
            slots = [(xblk, h) for h in range(HPC) for xblk in range(NXB)]

            # projection work units, pulled one per logits group inside
            # phase_a so their cast-WAR stalls hide under the ACT-paced
            # stream instead of head-of-line blocking the PE queue.
            proj_tasks = []

            def pull_task():
                if proj_tasks:
                    proj_tasks.pop(0)()

            def phase_a(s, tasks=0):
                xblk, h = slots[s]
                xs = slice(xblk * 512, (xblk + 1) * 512)
                E = [
                    epool.tile([128, 2, 512], BF, tag="E", name=f"E_{s}_{g}")
                    for g in range(8)
                ]
                for g in range(8):
                    pl = plpool.tile([128, 1024], F32, tag="pl", name=f"pl_{s}_{g}")
                    for j in range(2):
                        yt = 2 * g + j
                        nc.tensor.matmul(
                            pl[:, j * 512 : (j + 1) * 512],
                            K[:, h, yt * 128 : (yt + 1) * 128],
                            Q[:, h, xs],
                            start=True,
                            stop=True,
                        )
                    nc.scalar.activation(E[g][:], pl[:], EXPF, scale=SCALE)
                    if tasks:
                        pull_task()
                        tasks -= 1
                return E

            def phase_b(s, E):
                xblk, h = slots[s]
                for half in range(2):
                    po = popool.tile(
                        [128, 2, D + 1], F32, tag="po", name=f"po_{s}_{half}"
                    )
                    for sub in range(2):
                        sx = 2 * half + sub
                        for yt in range(NYT):
                            g, j = yt // 2, yt % 2
                            nc.tensor.matmul(
                                po[:, sub, :],
                                E[g][:, j, sx * 128 : (sx + 1) * 128],
                                VWT[:, yt, h, :],
                                start=(yt == 0),
                                stop=(yt == NYT - 1),
                            )
                    rc = rcpool.tile([128, 2], F32, tag="rc", name=f"rc_{s}_{half}")
                    nc.vector.reciprocal(rc[:], po[:, :, 128])
                    for sub in range(2):
                        sx = 2 * half + sub
                        a = ACC[:, xblk, sx, :]
                        if h == 0:
                            nc.vector.tensor_scalar(
                                a, po[:, sub, 0:D], rc[:, sub : sub + 1], None,
                                mybir.AluOpType.mult,
                            )
                        else:
                            nc.vector.scalar_tensor_tensor(
                                a, po[:, sub, 0:D], rc[:, sub : sub + 1], a,
                                mybir.AluOpType.mult, mybir.AluOpType.add,
                            )
                    if h == HPC - 1 and sub == 1:
                        orr = out.rearrange(
                            "(xb s p) o -> p xb s o", p=128, s=4
                        )
                        nc.sync.dma_start(
                            orr[:, xblk, 2 * half : 2 * half + 2, :],
                            ACC[:, xblk, 2 * half : 2 * half + 2, :],
                        )

            # ---- head: slot 0 interleaved with K-h0 projections -------------
            # (logits group g needs only K block g//2, so the exp stream
            # starts as soon as one K block and the slot-0 Q slice landed)
            E0 = [
                epool.tile([128, 2, 512], BF, tag="E", name=f"E_0_{g}")
                for g in range(8)
            ]

            def a0_pair(nb):
                for g in (2 * nb, 2 * nb + 1):
                    pl = plpool.tile([128, 1024], F32, tag="pl", name=f"pl_0_{g}")
                    for j in range(2):
                        yt = 2 * g + j
                        nc.tensor.matmul(
                            pl[:, j * 512 : (j + 1) * 512],
                            K[:, 0, yt * 128 : (yt + 1) * 128],
                            Q[:, 0, 0:512],
                            start=True,
                            stop=True,
                        )
                    nc.scalar.activation(E0[g][:], pl[:], EXPF, scale=SCALE)

            proj_k(0, 0, 1)
            proj_q(0, 0, 1)
            a0_pair(0)
            proj_k(0, 1, 2)
            a0_pair(1)
            proj_k(0, 2, 3)
            a0_pair(2)
            proj_k(0, 3, 4)
            a0_pair(3)
            proj_q(0, 1, NXB)

            def _kq_tasks(h):
                return [
                    (lambda h=h, nb=nb: proj_k(h, nb, nb + 1)) for nb in range(NXB)
                ] + [
                    (lambda h=h, nb=nb: proj_q(h, nb, nb + 1)) for nb in range(NXB)
                ]

            proj_tasks += [
                (lambda yt=yt: proj_vw(yt, yt + 1)) for yt in range(NYT)
            ]
            # two-deep pipeline: A(s) runs two slots ahead of B(s-2), so B
            # never head-of-line blocks the ACT-feeding logits stream.
            Es = {0: E0}
            Es[1] = phase_a(1, tasks=8)
            Es[2] = phase_a(2, tasks=8)
            phase_b(0, Es.pop(0))
            for s in range(3, len(slots)):
                if s == 3:
                    proj_tasks.extend(_kq_tasks(1))
                    ntask = 8
                elif s == 5:
                    proj_tasks.extend(_kq_tasks(2))
                    ntask = 4
                elif s == 6:
                    ntask = 4
                elif s == 9:
                    proj_tasks.extend(_kq_tasks(3))
                    ntask = 4
                elif s == 10:
                    ntask = 4
                else:
                    ntask = 0
                Es[s] = phase_a(s, tasks=ntask)
                phase_b(s - 2, Es.pop(s - 2))
            phase_b(len(slots) - 2, Es.pop(len(slots) - 2))
            phase_b(len(slots) - 1, Es.pop(len(slots) - 1))

    _split_multi_waits(nc)
    return nc


_NC = None


def _get_nc():
    global _NC
    if _NC is None:
        _NC = _build_nc()
    return _NC


def kernel(x, y, Wq, Wk, Wv, Wo):
    global LAST_EXEC_NS, LAST_RESULTS
    x = np.asarray(x, dtype=np.float32)
    y = np.asarray(y, dtype=np.float32)
    Wq3 = np.asarray(Wq, dtype=np.float32).reshape(H, D, C)
    Wk3 = np.asarray(Wk, dtype=np.float32).reshape(H, D, C)
    Wv3 = np.asarray(Wv, dtype=np.float32).reshape(H, D, C)
    Wo2 = np.asarray(Wo, dtype=np.float32)  # [D, H*D]

    bf16 = ml_dtypes.bfloat16

    in_maps = []
    for core in range(NCORES):
        b, g = core // 2, core % 2
        hsl = slice(4 * g, 4 * g + HPC)
        wqt = Wq3[hsl].reshape(HPC * D, C).T  # [c, hd]
        wkt = Wk3[hsl].reshape(HPC * D, C).T
        # VW_h = Wo_h @ Wv_h, stacked over the core's heads -> [c, hd]
        vw = np.stack(
            [
                Wo2[:, (4 * g + hh) * D : (4 * g + hh + 1) * D]
                @ Wv3[4 * g + hh]
                for hh in range(HPC)
            ]
        )  # [HPC, D, C]
        wvwt = vw.reshape(HPC * D, C).T
        wpack = np.concatenate([wkt, wqt, wvwt], axis=1)  # [c, 3*hd]
        in_maps.append(
            {
                "xb": np.ascontiguousarray(x[b]).astype(bf16),
                "yb": np.ascontiguousarray(y[b]).astype(bf16),
                "wpack": np.ascontiguousarray(wpack).astype(bf16),
            }
        )

    import os

    trace = bool(int(os.environ.get("ATTN_TRACE", "0")))
    res = run_bass_kernel_spmd(
        _get_nc(), in_maps, core_ids=list(range(NCORES)), trace=trace
    )
    LAST_EXEC_NS = res.exec_time_ns
    LAST_RESULTS = res

    outp = np.empty((B, D, N), dtype=np.float32)
    for b in range(B):
        outp[b] = (res.results[2 * b]["out"] + res.results[2 * b + 1]["out"]).T
    return outp
